# revision 1
# baseline (speedup 1.0000x reference)
"""Local (windowed) attention scores kernel for Trainium2, 8 NeuronCores.

Computes softmax(Q_win @ [K_prev|K_self|K_next]^T / sqrt(d)) per 128-wide
window, drops windows 2 and 34, zeros the padded edge regions of windows 0
and 63.  Data-parallel over the collapsed batch*heads axis (32 -> 4 per core).

Scheduling constraint discovered the hard way: walrus places every sync wait
of a Matmult on the LDWEIGHTS struct, which has a single wait slot -- so each
PE instruction may wait on at most ONE semaphore.  The kernel is therefore
structured so PE's only cross-engine dependency is DVE: tiny "absorber"
matmuls soak up each input-DMA wait, DVE produces every SBUF operand PE
reads, and DVE (not ACT) releases every PSUM slot by copying scores out.
"""

import sys

for _p in ("/opt/trn_rl_repo", "/opt/trn_rl_repo/concourse"):
    if _p not in sys.path:
        sys.path.insert(0, _p)

import numpy as np

B, H, N, D = 4, 8, 8192, 64
BH = B * H                      # 32
NCORES = 8
BHC = BH // NCORES              # 4 batch-heads per core
W = 128                         # window size
NW = N // W                     # 64 windows
EXCLUDED = (2, 34)
REMAINING = [i for i in range(NW) if i not in EXCLUDED]
NOUT = len(REMAINING)           # 62
J = 3 * W                       # 384 keys per query window
SCALE = float(D) ** -0.5        # 0.125

GS = 8                          # output windows per staging buffer / out-DMA
CH = 8                          # windows per input DMA chunk

_cached_nc = None


def _build():
    import concourse.bass as bass
    import concourse.mybir as mybir
    import concourse.tile as tile
    from concourse import bacc
    from concourse.masks import make_identity
    from concourse.tile import add_dep_helper

    fp32 = mybir.dt.float32
    nc = bacc.Bacc("TRN2", target_bir_lowering=False, debug=False)
    q = nc.dram_tensor("q", [BHC, N, D], fp32, kind="ExternalInput").ap()
    k = nc.dram_tensor("k", [BHC, N, D], fp32, kind="ExternalInput").ap()
    out = nc.dram_tensor("out", [BHC, NOUT, W, J], fp32, kind="ExternalOutput").ap()

    def raw(inst):
        return inst.ins if hasattr(inst, "ins") and not isinstance(inst.ins, list) else inst

    with tile.TileContext(nc) as tc:
        from contextlib import ExitStack

        with ExitStack() as ctx:
            singles = ctx.enter_context(tc.tile_pool(name="singles", bufs=1))
            qin_pool = ctx.enter_context(tc.tile_pool(name="qin", bufs=12))
            kin_pool = ctx.enter_context(tc.tile_pool(name="kin", bufs=12))
            kt_pool = ctx.enter_context(tc.tile_pool(name="kt", bufs=2))
            qt_pool = ctx.enter_context(tc.tile_pool(name="qt", bufs=6))
            stage_pool = ctx.enter_context(tc.tile_pool(name="stage", bufs=3))
            sums_pool = ctx.enter_context(tc.tile_pool(name="sums", bufs=4))
            tpsum = ctx.enter_context(tc.tile_pool(name="tpsum", bufs=4, space="PSUM"))
            spsum = ctx.enter_context(tc.tile_pool(name="spsum", bufs=3, space="PSUM"))
            scrapp = ctx.enter_context(tc.tile_pool(name="scrap", bufs=1, space="PSUM"))

            ident = singles.tile([128, 128], fp32)
            make_identity(nc, ident)
            scrap = scrapp.tile([2, 2], fp32, tag="scrap")
            # absorb the gpsimd (ident) wait into PE's clock once
            nc.tensor.matmul(scrap, ident[:, :2], ident[:, :2], start=True, stop=True)

            def absorber(chunk):
                """1-wait PE matmul absorbing `chunk`'s DMA completion."""
                return nc.tensor.matmul(
                    scrap, chunk[:, 0, :2], chunk[:, 0, :2], start=True, stop=True
                )

            for bh in range(BHC):
                # ---- load K/Q chunks (one tile per DMA) ----
                kchunks, qchunks = [], []
                for g in range(NW // CH):
                    kc = kin_pool.tile([W, CH, D], fp32, tag="kin")
                    src = k[bh, g * CH * W : (g + 1) * CH * W, :].rearrange(
                        "(w p) d -> p w d", p=W
                    )
                    nc.gpsimd.dma_start(out=kc, in_=src)
                    kchunks.append(kc)
                for g in range(NW // CH):
                    qc = qin_pool.tile([W, CH, D], fp32, tag="qin")
                    src = q[bh, g * CH * W : (g + 1) * CH * W, :].rearrange(
                        "(w p) d -> p w d", p=W
                    )
                    nc.gpsimd.dma_start(out=qc, in_=src)
                    qchunks.append(qc)

                # ---- transpose K into KT (64 x 8192) ----
                kt = kt_pool.tile([D, NW * W], fp32, tag="kt")
                for g in range(NW // CH):
                    ab = absorber(kchunks[g])
                    for wl in range(CH):
                        w = g * CH + wl
                        tp = tpsum.tile([D, W], fp32, tag="t")
                        mm = nc.tensor.matmul(
                            tp, kchunks[g][:, wl, :], ident, start=True, stop=True
                        )
                        add_dep_helper(raw(mm), raw(ab), False, "transpose after absorber")
                        nc.vector.tensor_copy(out=kt[:, w * W : (w + 1) * W], in_=tp)

                # ---- per output-window group ----
                o0 = 0
                q_absorbed = -1
                while o0 < NOUT:
                    gs = min(GS, NOUT - o0)
                    stage = stage_pool.tile([W, GS, J], fp32, tag="stage")
                    sums = sums_pool.tile([W, GS], fp32, tag="sums")
                    for oi in range(gs):
                        wi = REMAINING[o0 + oi]
                        g = wi // CH
                        if g != q_absorbed:
                            qab = absorber(qchunks[g])
                            q_absorbed = g
                        tpq = tpsum.tile([D, W], fp32, tag="t")
                        mmq = nc.tensor.matmul(
                            tpq, qchunks[g][:, wi % CH, :], ident,
                            start=True, stop=True,
                        )
                        add_dep_helper(raw(mmq), raw(qab), False, "transpose after absorber")
                        qt = qt_pool.tile([D, W], fp32, tag="qt")
                        nc.vector.tensor_copy(out=qt, in_=tpq)

                        sp = spsum.tile([W, J], fp32, tag="s")
                        if wi == 0:
                            # prev window padded: valid j = [W, 3W)
                            nc.tensor.matmul(
                                sp[:, :256], qt, kt[:, : 2 * W], start=True, stop=True
                            )
                            nc.vector.memset(stage[:, oi, :W], 0.0)
                            nc.vector.tensor_copy(
                                out=stage[:, oi, W:], in_=sp[:, :256]
                            )
                            nc.scalar.activation(
                                stage[:, oi, W:],
                                stage[:, oi, W:],
                                mybir.ActivationFunctionType.Exp,
                                scale=SCALE,
                                accum_out=sums[:, oi : oi + 1],
                            )
                        elif wi == NW - 1:
                            # next window padded: valid j = [0, 2W)
                            nc.tensor.matmul(
                                sp[:, :256], qt, kt[:, (NW - 2) * W :],
                                start=True, stop=True,
                            )
                            nc.vector.memset(stage[:, oi, 2 * W :], 0.0)
                            nc.vector.tensor_copy(
                                out=stage[:, oi, : 2 * W], in_=sp[:, :256]
                            )
                            nc.scalar.activation(
                                stage[:, oi, : 2 * W],
                                stage[:, oi, : 2 * W],
                                mybir.ActivationFunctionType.Exp,
                                scale=SCALE,
                                accum_out=sums[:, oi : oi + 1],
                            )
                        else:
                            nc.tensor.matmul(
                                sp, qt, kt[:, (wi - 1) * W : (wi + 2) * W],
                                start=True, stop=True,
                            )
                            nc.vector.tensor_copy(out=stage[:, oi, :], in_=sp)
                            nc.scalar.activation(
                                stage[:, oi, :],
                                stage[:, oi, :],
                                mybir.ActivationFunctionType.Exp,
                                scale=SCALE,
                                accum_out=sums[:, oi : oi + 1],
                            )

                    recip = sums_pool.tile([W, GS], fp32, tag="recip")
                    nc.vector.reciprocal(recip[:, :gs], sums[:, :gs])
                    for oi in range(gs):
                        # normalize on ACT: out = Copy(in * recip)
                        nc.scalar.mul(
                            stage[:, oi, :], stage[:, oi, :], recip[:, oi : oi + 1]
                        )
                    dst = out[bh, o0 : o0 + gs].rearrange("w i j -> i w j")
                    nc.gpsimd.dma_start(out=dst, in_=stage[:, :gs, :])
                    o0 += gs
    nc.compile()
    return nc


def _run(q, k, trace=False):
    from concourse.bass_utils import run_bass_kernel_spmd

    global _cached_nc
    if _cached_nc is None:
        _cached_nc = _build()
    nc = _cached_nc

    q = np.ascontiguousarray(np.asarray(q), dtype=np.float32).reshape(BH, N, D)
    k = np.ascontiguousarray(np.asarray(k), dtype=np.float32).reshape(BH, N, D)
    in_maps = [
        {
            "q": np.ascontiguousarray(q[c * BHC : (c + 1) * BHC]),
            "k": np.ascontiguousarray(k[c * BHC : (c + 1) * BHC]),
        }
        for c in range(NCORES)
    ]
    res = run_bass_kernel_spmd(nc, in_maps, core_ids=list(range(NCORES)), trace=trace)
    full = np.concatenate([res.results[c]["out"] for c in range(NCORES)], axis=0)
    return full.reshape(BH, NOUT, W, J), res


def kernel(q, k):
    out, _ = _run(q, k, trace=False)
    return out



# revision 7
# speedup vs baseline: 1.2704x; 1.2704x over previous
"""Local (windowed) attention scores kernel for Trainium2, 8 NeuronCores.

Computes softmax(Q_win @ [K_prev|K_self|K_next]^T / sqrt(d)) per 128-wide
window, drops windows 2 and 34, zeros the padded edge regions of windows 0
and 63.  Data-parallel over the collapsed batch*heads axis (32 -> 4 per core).

Design notes (v2, rebuilt around the engine-busy profile of v1):
  * All matmuls run in bf16 (1 cyc/row on PE vs 4 for fp32).  GPSIMD does
    the fp32->bf16 input conversion; it is otherwise idle.
  * Inputs are loaded with fully contiguous HBM reads (16KB per partition:
    partition p holds tokens [64p, 64p+64)).  The resulting Q^T/K^T tiles
    are "a-major": column (a, p) = token 64p+a.  The score matmuls use
    strided APs over those tiles, so the output rows/cols come out in a
    fixed permutation that the host undoes for free.
  * ACT only does exp, two windows per instruction, reading score PSUM
    directly and writing a bf16 stage buffer.
  * DVE computes the softmax denominators with a copy+accum pass and
    normalizes with per-window tensor_scalar multiplies; both run in the
    packed-bf16 fast mode.
  * Output is written to HBM in bf16 (halves the dominant DMA stream) and
    upcast to fp32 on the host.

Scheduling constraint carried over from v1: walrus places every sync wait
of a Matmult on the LDWEIGHTS struct, which has a single wait slot -- each
PE instruction may wait on at most ONE semaphore.  Tiny "absorber" matmuls
soak the Pool(convert)/DVE(transpose-copy) waits so that every real PE
instruction carries at most one cross-engine wait (DVE for transposes via
psum-recycle, ACT for score matmuls via psum-recycle).
"""

import sys

for _p in ("/opt/trn_rl_repo", "/opt/trn_rl_repo/concourse"):
    if _p not in sys.path:
        sys.path.insert(0, _p)

import numpy as np

B, H, N, D = 4, 8, 8192, 64
BH = B * H                      # 32
NCORES = 8
BHC = BH // NCORES              # 4 batch-heads per core
W = 128                         # window size
NW = N // W                     # 64 windows
EXCLUDED = (2, 34)
REMAINING = [i for i in range(NW) if i not in EXCLUDED]
NOUT = len(REMAINING)           # 62
J = 3 * W                       # 384 keys per query window
SCALE = float(D) ** -0.5        # 0.125

GS = 6                          # output windows per stage buffer / out-DMA
TA = 8                          # transpose slots per PSUM tile (1 bank bf16)

_cached_nc = None


def _build():
    import concourse.bass as bass
    import concourse.mybir as mybir
    import concourse.tile as tile
    from concourse import bacc
    from concourse.masks import make_identity
    from concourse.tile import add_dep_helper

    fp32 = mybir.dt.float32
    bf16 = mybir.dt.bfloat16
    mult = mybir.AluOpType.mult

    nc = bacc.Bacc("TRN2", target_bir_lowering=False, debug=False)
    q = nc.dram_tensor("q", [BHC, N, D], fp32, kind="ExternalInput").ap()
    k = nc.dram_tensor("k", [BHC, N, D], fp32, kind="ExternalInput").ap()
    out = nc.dram_tensor("out", [BHC, NOUT, W, J], bf16, kind="ExternalOutput").ap()

    def raw(inst):
        return inst.ins if hasattr(inst, "ins") and not isinstance(inst.ins, list) else inst

    with tile.TileContext(nc) as tc:
        from contextlib import ExitStack

        with ExitStack() as ctx:
            singles = ctx.enter_context(tc.tile_pool(name="singles", bufs=1))
            kin_pool = ctx.enter_context(tc.tile_pool(name="kin", bufs=2))
            qin_pool = ctx.enter_context(tc.tile_pool(name="qin", bufs=2))
            kbf_pool = ctx.enter_context(tc.tile_pool(name="kbf", bufs=2))
            qbf_pool = ctx.enter_context(tc.tile_pool(name="qbf", bufs=2))
            kt_pool = ctx.enter_context(tc.tile_pool(name="kt", bufs=2))
            qt_pool = ctx.enter_context(tc.tile_pool(name="qt", bufs=2))
            sa_pool = ctx.enter_context(tc.tile_pool(name="stageA", bufs=3))
            sb_pool = ctx.enter_context(tc.tile_pool(name="stageB", bufs=2))
            sums_pool = ctx.enter_context(tc.tile_pool(name="sums", bufs=2))
            tpsum = ctx.enter_context(tc.tile_pool(name="tpsum", bufs=2, space="PSUM"))
            spsum = ctx.enter_context(tc.tile_pool(name="spsum", bufs=2, space="PSUM"))
            scrapp = ctx.enter_context(tc.tile_pool(name="scrap", bufs=1, space="PSUM"))

            identb = singles.tile([128, 128], bf16)
            make_identity(nc, identb)
            scrap = scrapp.tile([2, 2], fp32, tag="scrap")
            # absorb the gpsimd (ident) wait into PE's clock once
            nc.tensor.matmul(scrap, identb[:, :2], identb[:, :2], start=True, stop=True)

            def absorber(lhs2, rhs2, dep=None, why="absorber"):
                """1-wait PE matmul absorbing a cross-engine dependency."""
                mm = nc.tensor.matmul(scrap, lhs2, rhs2, start=True, stop=True)
                if dep is not None:
                    add_dep_helper(raw(mm), raw(dep), False, why)
                return mm

            for bh in range(BHC):
                # ---- contiguous loads: partition p <- tokens [64p, 64p+64) ----
                ktile = kin_pool.tile([128, 64, D], fp32, tag="kin")
                qtile = qin_pool.tile([128, 64, D], fp32, tag="qin")
                nc.sync.dma_start(out=ktile, in_=k[bh].rearrange("(p a) d -> p a d", p=128))
                nc.sync.dma_start(out=qtile, in_=q[bh].rearrange("(p a) d -> p a d", p=128))

                # ---- fp32 -> bf16 on GPSIMD ----
                kbf = kbf_pool.tile([128, 64, D], bf16, tag="kbf")
                qbf = qbf_pool.tile([128, 64, D], bf16, tag="qbf")
                nc.gpsimd.tensor_copy(out=kbf, in_=ktile)
                nc.gpsimd.tensor_copy(out=qbf, in_=qtile)

                # ---- transpose K^T a-major (kt[d, a, p] = K[64p+a, d]) and
                # ---- Q^T token-major (qt[d, n] = Q[n, d]; strided copy) ----
                kt = kt_pool.tile([D, 64, 128], bf16, tag="kt")
                qt = qt_pool.tile([D, N], bf16, tag="qt")
                qt_pa = qt.rearrange("d (p a) -> d p a", p=128)
                for src, dst, is_q in ((kbf, kt, False), (qbf, qt, True)):
                    ab = absorber(src[:, 0, :2], identb[:, :2], dep=None)
                    first = True
                    for a0 in range(0, 64, TA):
                        tp = tpsum.tile([D, TA, 128], bf16, tag="t")
                        for t in range(TA):
                            mm = nc.tensor.transpose(tp[:, t, :], src[:, a0 + t, :], identb)
                            if first:
                                add_dep_helper(raw(mm), raw(ab), False, "transpose after absorber")
                                first = False
                        if is_q:
                            nc.vector.tensor_copy(
                                out=qt_pa[:, :, a0 : a0 + TA],
                                in_=tp.rearrange("d t p -> d p t"),
                            )
                        else:
                            nc.vector.tensor_copy(out=dst[:, a0 : a0 + TA, :], in_=tp)

                # one absorber soaking the DVE tick of the kt/qt copies so the
                # score matmuls' only cross-engine wait is the ACT psum-recycle
                absorber(kt[:, 0, :2], qt[:, :2], dep=None)

                # ---- per output-window group ----
                o0 = 0
                while o0 < NOUT:
                    gs = min(GS, NOUT - o0)
                    stage_a = sa_pool.tile([128, GS, J], bf16, tag="sa")
                    stage_b = sb_pool.tile([128, GS, J], bf16, tag="sb")
                    sums = sums_pool.tile([128, GS], fp32, tag="sums")
                    recip = sums_pool.tile([128, GS], fp32, tag="recip")
                    lens = []
                    for p0 in range(0, gs, 2):
                        sc = spsum.tile([128, 2, 512], fp32, tag="s")
                        plens = []
                        for s2 in range(2):
                            s = p0 + s2
                            wi = REMAINING[o0 + s]
                            lo = max(0, 2 * wi - 2)
                            hi = min(128, 2 * wi + 4)
                            cols = 64 * (hi - lo)
                            plens.append(cols)
                            lens.append(cols)
                            nc.tensor.matmul(
                                sc[:, s2, :cols],
                                qt[:, wi * W : (wi + 1) * W],
                                kt[:, :, lo:hi],
                                start=True,
                                stop=True,
                            )
                        # exp on ACT straight out of PSUM into the bf16 stage
                        if plens[0] == plens[1] == J:
                            nc.scalar.activation(
                                stage_a[:, p0 : p0 + 2, :],
                                sc[:, :, :J],
                                mybir.ActivationFunctionType.Exp,
                                scale=SCALE,
                            )
                        else:
                            for s2 in range(2):
                                nc.scalar.activation(
                                    stage_a[:, p0 + s2, : plens[s2]],
                                    sc[:, s2, : plens[s2]],
                                    mybir.ActivationFunctionType.Exp,
                                    scale=SCALE,
                                )
                        # denominators: copy+accum on DVE (packed bf16 fast mode)
                        for s2 in range(2):
                            s = p0 + s2
                            nc.vector.tensor_scalar(
                                out=stage_b[:, s, : lens[s]],
                                in0=stage_a[:, s, : lens[s]],
                                scalar1=1.0,
                                scalar2=None,
                                op0=mult,
                                op1=mybir.AluOpType.add,
                                accum_out=sums[:, s : s + 1],
                            )
                    nc.vector.reciprocal(recip[:, :gs], sums[:, :gs])
                    for s in range(gs):
                        nc.vector.tensor_scalar(
                            out=stage_a[:, s, : lens[s]],
                            in0=stage_b[:, s, : lens[s]],
                            scalar1=recip[:, s : s + 1],
                            scalar2=None,
                            op0=mult,
                        )
                    dst = out[bh, o0 : o0 + gs].rearrange("w c j -> c w j")
                    nc.sync.dma_start(out=dst, in_=stage_a[:, :gs, :])
                    o0 += gs
    nc.compile()
    return nc


# ---- host-side permutation maps -------------------------------------------
# Output rows are already in query order.  Stage col a*6+dp holds key token
# 64*(2(w-1)+dp)+a, i.e. j_ref = 64*dp+a -> col(j) = (j%64)*6 + j//64.
# Window 0 (4 p-slots, j_ref>=128): col = ((j-128)%64)*4 + (j-128)//64.
# Window 63 (4 p-slots, j_ref<256): col = (j%64)*4 + j//64.
_JM = ((np.arange(J) % 64) * 6 + np.arange(J) // 64).astype(np.intp)
_J0 = (((np.arange(128, J) - 128) % 64) * 4 + (np.arange(128, J) - 128) // 64).astype(np.intp)
_J63 = ((np.arange(256) % 64) * 4 + np.arange(256) // 64).astype(np.intp)


def _assemble(raw):
    """raw: [BH, NOUT, 128, 384] bf16 device layout -> fp32 reference layout."""
    res = np.empty((BH, NOUT, W, J), np.float32)
    res[:, 1 : NOUT - 1] = raw[:, 1 : NOUT - 1][..., _JM]
    res[:, 0, :, :128] = 0.0
    res[:, 0, :, 128:] = raw[:, 0][..., _J0]
    res[:, NOUT - 1, :, :256] = raw[:, NOUT - 1][..., _J63]
    res[:, NOUT - 1, :, 256:] = 0.0
    return res


def _run(q, k, trace=False):
    from concourse.bass_utils import run_bass_kernel_spmd

    global _cached_nc
    if _cached_nc is None:
        _cached_nc = _build()
    nc = _cached_nc

    q = np.ascontiguousarray(np.asarray(q), dtype=np.float32).reshape(BH, N, D)
    k = np.ascontiguousarray(np.asarray(k), dtype=np.float32).reshape(BH, N, D)
    in_maps = [
        {
            "q": np.ascontiguousarray(q[c * BHC : (c + 1) * BHC]),
            "k": np.ascontiguousarray(k[c * BHC : (c + 1) * BHC]),
        }
        for c in range(NCORES)
    ]
    res = run_bass_kernel_spmd(nc, in_maps, core_ids=list(range(NCORES)), trace=trace)
    raw = np.concatenate([np.asarray(res.results[c]["out"]) for c in range(NCORES)], axis=0)
    return _assemble(raw), res


def kernel(q, k):
    out, _ = _run(q, k, trace=False)
    return out


# revision 8
# speedup vs baseline: 1.2764x; 1.0048x over previous
"""Local (windowed) attention scores kernel for Trainium2, 8 NeuronCores.

Computes softmax(Q_win @ [K_prev|K_self|K_next]^T / sqrt(d)) per 128-wide
window, drops windows 2 and 34, zeros the padded edge regions of windows 0
and 63.  Data-parallel over the collapsed batch*heads axis (32 -> 4 per core).

Design (v3):
  * All device math in fp16 (PE 1 cyc/row, DVE 2x/4x packed modes, 8x the
    mantissa of bf16).  GPSIMD casts the fp32 inputs.
  * Inputs loaded with fully contiguous HBM reads (16KB/partition; partition
    p holds tokens [64p, 64p+64)).  K^T is kept "a-major" (column (a,p) =
    token 64p+a) and the score matmuls use a strided moving AP over it, so
    output columns come out in a fixed permutation undone on the host.
    Q^T is stored token-major (strided DVE copy) because the stationary
    operand must have a single free dim.
  * ACT does exp straight out of score-PSUM into an fp16 stage (2 windows
    per instruction), plus the K^T PSUM->SBUF copies and 1-in-3 group
    normalizes (load balance with DVE).
  * Softmax denominators come from a pairwise tensor_tensor fold tree on
    DVE (2x packed mode) + one segmented 1x tail reduce -- the per-window
    accum-reduce op only has a 1x uop and was the previous bottleneck.
  * Output written to HBM in fp16 (halves the dominant DMA stream) and
    upcast on the host.

Scheduling constraint: walrus places every sync wait of a Matmult on the
LDWEIGHTS struct, which has a single wait slot -- each PE instruction may
wait on at most ONE semaphore.  Tiny "absorber" matmuls soak the
Pool(cast)/DVE(q-copies)/ACT(k-copies) ticks so every real PE instruction
carries at most one cross-engine wait.
"""

import sys

for _p in ("/opt/trn_rl_repo", "/opt/trn_rl_repo/concourse"):
    if _p not in sys.path:
        sys.path.insert(0, _p)

import numpy as np

B, H, N, D = 4, 8, 8192, 64
BH = B * H                      # 32
NCORES = 8
BHC = BH // NCORES              # 4 batch-heads per core
W = 128                         # window size
NW = N // W                     # 64 windows
EXCLUDED = (2, 34)
REMAINING = [i for i in range(NW) if i not in EXCLUDED]
NOUT = len(REMAINING)           # 62
J = 3 * W                       # 384 keys per query window
SCALE = float(D) ** -0.5        # 0.125

GS = 6                          # output windows per stage buffer / out-DMA
TA = 8                          # transpose slots per PSUM tile (1 bank fp16)

_cached_nc = None


def _build():
    import concourse.bass as bass
    import concourse.mybir as mybir
    import concourse.tile as tile
    from concourse import bacc
    from concourse.masks import make_identity
    from concourse.tile import add_dep_helper

    fp32 = mybir.dt.float32
    fp16 = mybir.dt.float16
    mult = mybir.AluOpType.mult
    add = mybir.AluOpType.add

    nc = bacc.Bacc("TRN2", target_bir_lowering=False, debug=False)
    q = nc.dram_tensor("q", [BHC, N, D], fp32, kind="ExternalInput").ap()
    k = nc.dram_tensor("k", [BHC, N, D], fp32, kind="ExternalInput").ap()
    out = nc.dram_tensor("out", [BHC, NOUT, W, J], fp16, kind="ExternalOutput").ap()

    def raw(inst):
        return inst.ins if hasattr(inst, "ins") and not isinstance(inst.ins, list) else inst

    with tile.TileContext(nc) as tc:
        from contextlib import ExitStack

        with ExitStack() as ctx:
            singles = ctx.enter_context(tc.tile_pool(name="singles", bufs=1))
            kin_pool = ctx.enter_context(tc.tile_pool(name="kin", bufs=2))
            qin_pool = ctx.enter_context(tc.tile_pool(name="qin", bufs=2))
            kbf_pool = ctx.enter_context(tc.tile_pool(name="kbf", bufs=2))
            qbf_pool = ctx.enter_context(tc.tile_pool(name="qbf", bufs=2))
            kt_pool = ctx.enter_context(tc.tile_pool(name="kt", bufs=2))
            qt_pool = ctx.enter_context(tc.tile_pool(name="qt", bufs=2))
            sa_pool = ctx.enter_context(tc.tile_pool(name="stageA", bufs=3))
            sb_pool = ctx.enter_context(tc.tile_pool(name="stageB", bufs=2))
            sums_pool = ctx.enter_context(tc.tile_pool(name="sums", bufs=2))
            tpsum = ctx.enter_context(tc.tile_pool(name="tpsum", bufs=2, space="PSUM"))
            spsum = ctx.enter_context(tc.tile_pool(name="spsum", bufs=2, space="PSUM"))
            scrapp = ctx.enter_context(tc.tile_pool(name="scrap", bufs=1, space="PSUM"))

            identh = singles.tile([128, 128], fp16)
            make_identity(nc, identh)
            scrap = scrapp.tile([2, 2], fp32, tag="scrap")
            # absorb the gpsimd (ident) wait into PE's clock once
            nc.tensor.matmul(scrap, identh[:, :2], identh[:, :2], start=True, stop=True)

            def absorber(lhs2, rhs2, dep=None, why="absorber"):
                """1-wait PE matmul absorbing a cross-engine dependency."""
                mm = nc.tensor.matmul(scrap, lhs2, rhs2, start=True, stop=True)
                if dep is not None:
                    add_dep_helper(raw(mm), raw(dep), False, why)
                return mm

            norm_rr = 0  # round-robin for ACT/DVE normalize split
            for bh in range(BHC):
                # ---- contiguous loads: partition p <- tokens [64p, 64p+64) ----
                ktile = kin_pool.tile([128, 64, D], fp32, tag="kin")
                qtile = qin_pool.tile([128, 64, D], fp32, tag="qin")
                nc.sync.dma_start(out=ktile, in_=k[bh].rearrange("(p a) d -> p a d", p=128))
                nc.sync.dma_start(out=qtile, in_=q[bh].rearrange("(p a) d -> p a d", p=128))

                # ---- fp32 -> fp16 on GPSIMD ----
                kbf = kbf_pool.tile([128, 64, D], fp16, tag="kbf")
                qbf = qbf_pool.tile([128, 64, D], fp16, tag="qbf")
                nc.gpsimd.tensor_copy(out=kbf, in_=ktile)
                nc.gpsimd.tensor_copy(out=qbf, in_=qtile)

                # ---- transpose K^T a-major (kt[d, a, p] = K[64p+a, d], ACT copies)
                # ---- and Q^T token-major (qt[d, n] = Q[n, d]; strided DVE copy) ----
                kt = kt_pool.tile([D, 64, 128], fp16, tag="kt")
                qt = qt_pool.tile([D, N], fp16, tag="qt")
                qt_pa = qt.rearrange("d (p a) -> d p a", p=128)
                for src, dst, is_q in ((kbf, kt, False), (qbf, qt, True)):
                    ab = absorber(src[:, 0, :2], identh[:, :2], dep=None)
                    first = True
                    for a0 in range(0, 64, TA):
                        tp = tpsum.tile([D, TA, 128], fp16, tag="t")
                        for t in range(TA):
                            mm = nc.tensor.transpose(tp[:, t, :], src[:, a0 + t, :], identh)
                            if first:
                                add_dep_helper(raw(mm), raw(ab), False, "transpose after absorber")
                                first = False
                        if is_q:
                            nc.vector.tensor_copy(
                                out=qt_pa[:, :, a0 : a0 + TA],
                                in_=tp.rearrange("d t p -> d p t"),
                            )
                        else:
                            nc.scalar.copy(out=dst[:, a0 : a0 + TA, :], in_=tp)

                # absorbers soaking the DVE (qt) and ACT (kt) copy ticks so the
                # score matmuls' only cross-engine wait is the ACT psum-recycle
                absorber(kt[:, 0, :2], identh[:64, :2], dep=None)
                absorber(qt[:64, :2], identh[:64, :2], dep=None)

                # ---- per output-window group ----
                o0 = 0
                while o0 < NOUT:
                    gs = min(GS, NOUT - o0)
                    stage_a = sa_pool.tile([128, GS, J], fp16, tag="sa")
                    stage_b = sb_pool.tile([128, GS, J], fp16, tag="sb")
                    sums = sums_pool.tile([128, GS], fp32, tag="sums")
                    recip = sums_pool.tile([128, GS], fp32, tag="recip")
                    lens = []
                    for p0 in range(0, gs, 2):
                        sc = spsum.tile([128, 2, 512], fp32, tag="s")
                        plens = []
                        for s2 in range(2):
                            s = p0 + s2
                            wi = REMAINING[o0 + s]
                            lo = max(0, 2 * wi - 2)
                            hi = min(128, 2 * wi + 4)
                            cols = 64 * (hi - lo)
                            plens.append(cols)
                            lens.append(cols)
                            nc.tensor.matmul(
                                sc[:, s2, :cols],
                                qt[:, wi * W : (wi + 1) * W],
                                kt[:, :, lo:hi],
                                start=True,
                                stop=True,
                            )
                        # exp on ACT straight out of PSUM into the fp16 stage
                        if plens[0] == plens[1] == J:
                            nc.scalar.activation(
                                stage_a[:, p0 : p0 + 2, :],
                                sc[:, :, :J],
                                mybir.ActivationFunctionType.Exp,
                                scale=SCALE,
                            )
                        else:
                            for s2 in range(2):
                                nc.scalar.activation(
                                    stage_a[:, p0 + s2, : plens[s2]],
                                    sc[:, s2, : plens[s2]],
                                    mybir.ActivationFunctionType.Exp,
                                    scale=SCALE,
                                )
                                if plens[s2] < J:
                                    # zero the tail so the fold tree sums stay exact
                                    nc.vector.memset(stage_a[:, p0 + s2, plens[s2] :], 0.0)

                    # ---- denominators: pairwise fold tree on DVE (2x packed) ----
                    # level outputs live in stage_b at per-slot offsets:
                    #   L1 [0:192], L2 [192:288], L3 [288:336], L4 [336:360], L5 [360:372]
                    a_v = stage_a[:, :gs]
                    b_v = stage_b[:, :gs]
                    nc.vector.tensor_tensor(
                        out=b_v[:, :, 0:192], in0=a_v[:, :, 0:192],
                        in1=a_v[:, :, 192:384], op=add)
                    nc.vector.tensor_tensor(
                        out=b_v[:, :, 192:288], in0=b_v[:, :, 0:96],
                        in1=b_v[:, :, 96:192], op=add)
                    nc.vector.tensor_tensor(
                        out=b_v[:, :, 288:336], in0=b_v[:, :, 192:240],
                        in1=b_v[:, :, 240:288], op=add)
                    nc.vector.tensor_tensor(
                        out=b_v[:, :, 336:360], in0=b_v[:, :, 288:312],
                        in1=b_v[:, :, 312:336], op=add)
                    nc.vector.tensor_tensor(
                        out=b_v[:, :, 360:372], in0=b_v[:, :, 336:348],
                        in1=b_v[:, :, 348:360], op=add)
                    nc.vector.tensor_reduce(
                        out=sums[:, :gs], in_=b_v[:, :, 360:372],
                        axis=mybir.AxisListType.X, op=add)
                    nc.vector.reciprocal(recip[:, :gs], sums[:, :gs])

                    # ---- normalize A -> B (split DVE/ACT for balance) ----
                    for s in range(gs):
                        on_act = s == 0 and norm_rr % 3 != 2
                        if on_act:
                            nc.scalar.mul(
                                stage_b[:, s, :], stage_a[:, s, :], recip[:, s : s + 1])
                        else:
                            nc.vector.tensor_scalar(
                                out=stage_b[:, s, :],
                                in0=stage_a[:, s, :],
                                scalar1=recip[:, s : s + 1],
                                scalar2=None,
                                op0=mult,
                            )
                    norm_rr += 1
                    dst = out[bh, o0 : o0 + gs].rearrange("w c j -> c w j")
                    nc.sync.dma_start(out=dst, in_=stage_b[:, :gs, :])
                    o0 += gs
    nc.compile()
    return nc


# ---- host-side permutation maps -------------------------------------------
# Output rows are already in query order.  Stage col a*6+dp holds key token
# 64*(2(w-1)+dp)+a, i.e. j_ref = 64*dp+a -> col(j) = (j%64)*6 + j//64.
# Window 0 (4 p-slots, j_ref>=128): col = ((j-128)%64)*4 + (j-128)//64.
# Window 63 (4 p-slots, j_ref<256): col = (j%64)*4 + j//64.
_JM = ((np.arange(J) % 64) * 6 + np.arange(J) // 64).astype(np.intp)
_J0 = (((np.arange(128, J) - 128) % 64) * 4 + (np.arange(128, J) - 128) // 64).astype(np.intp)
_J63 = ((np.arange(256) % 64) * 4 + np.arange(256) // 64).astype(np.intp)


def _assemble(raw):
    """raw: [BH, NOUT, 128, 384] fp16 device layout -> fp32 reference layout."""
    res = np.empty((BH, NOUT, W, J), np.float32)
    res[:, 1 : NOUT - 1] = raw[:, 1 : NOUT - 1][..., _JM]
    res[:, 0, :, :128] = 0.0
    res[:, 0, :, 128:] = raw[:, 0][..., _J0]
    res[:, NOUT - 1, :, :256] = raw[:, NOUT - 1][..., _J63]
    res[:, NOUT - 1, :, 256:] = 0.0
    return res


def _run(q, k, trace=False):
    from concourse.bass_utils import run_bass_kernel_spmd

    global _cached_nc
    if _cached_nc is None:
        _cached_nc = _build()
    nc = _cached_nc

    q = np.ascontiguousarray(np.asarray(q), dtype=np.float32).reshape(BH, N, D)
    k = np.ascontiguousarray(np.asarray(k), dtype=np.float32).reshape(BH, N, D)
    in_maps = [
        {
            "q": np.ascontiguousarray(q[c * BHC : (c + 1) * BHC]),
            "k": np.ascontiguousarray(k[c * BHC : (c + 1) * BHC]),
        }
        for c in range(NCORES)
    ]
    res = run_bass_kernel_spmd(nc, in_maps, core_ids=list(range(NCORES)), trace=trace)
    raw = np.concatenate([np.asarray(res.results[c]["out"]) for c in range(NCORES)], axis=0)
    return _assemble(raw), res


def kernel(q, k):
    out, _ = _run(q, k, trace=False)
    return out
